# revision 1
# baseline (speedup 1.0000x reference)
"""Trainium2 Bass kernel: multi-head attention (B=4, N=1024, D=1024, H=16)
distributed over 8 NeuronCores.

kernel(**inputs) takes the FULL inputs (x, w_qkv, w_out, b_out) as numpy
arrays, shards them (batch, query-half) -> one core each, runs an SPMD Bass
kernel on cores 0-7 and reassembles the full [4, 1024, 1024] fp32 output.
"""

import numpy as np
import concourse.bacc as bacc
import concourse.mybir as mybir
import concourse.tile as tile

dt = mybir.dt
F32, BF16 = dt.float32, dt.bfloat16

B, N, D = 4, 1024, 1024
H, DH = 16, 64
NQ = 512            # queries per core
P = 128
DC = D // P         # 8 contraction chunks
NT = N // P         # 8 key-token tiles
ET = 8              # feature tiles per q/k section
SCALE = DH ** -0.5
AF = mybir.ActivationFunctionType


def _build_nc():
    nc = bacc.Bacc("TRN2", target_bir_lowering=False, debug=False)
    xkT = nc.dram_tensor("xkT", [D, N], BF16, kind="ExternalInput")
    xqT = nc.dram_tensor("xqT", [D, NQ], BF16, kind="ExternalInput")
    wqkvT = nc.dram_tensor("wqkvT", [D, 3 * D], BF16, kind="ExternalInput")
    woutT = nc.dram_tensor("woutT", [D, D], BF16, kind="ExternalInput")
    bout = nc.dram_tensor("bout", [1, D], BF16, kind="ExternalInput")
    y = nc.dram_tensor("y", [NQ, D], F32, kind="ExternalOutput")

    with tile.TileContext(nc) as tc:
        with (
            tc.tile_pool(name="const", bufs=1) as cp,
            tc.tile_pool(name="work", bufs=2) as wp,
            tc.tile_pool(name="ps", bufs=1, space="PSUM") as pp,
        ):
            xk_sb = cp.tile([P, DC, N], BF16)
            xq_sb = cp.tile([P, DC, NQ], BF16)
            wqkv_sb = cp.tile([P, DC, 3 * D], BF16)
            wout_sb = cp.tile([P, DC, D], BF16)
            bout_sb = cp.tile([1, D], BF16)
            # DMA order = consumption order: q-proj operands first.
            # Single large strided transfers use the full DMA ring fan-out.
            nc.sync.dma_start(xq_sb[:, :, :],
                              xqT.ap().rearrange("(c p) n -> p c n", p=P))
            nc.sync.dma_start(
                wqkv_sb[:, :, 0:D],
                wqkvT.ap()[:, 0:D].rearrange("(c p) n -> p c n", p=P))
            for c in range(DC):
                nc.sync.dma_start(xk_sb[:, c, :], xkT.ap()[c * P:(c + 1) * P, :])
            for c in range(DC):
                nc.sync.dma_start(wqkv_sb[:, c, D:2 * D],
                                  wqkvT.ap()[c * P:(c + 1) * P, D:2 * D])
            for c in range(DC):
                nc.sync.dma_start(wqkv_sb[:, c, 2 * D:3 * D],
                                  wqkvT.ap()[c * P:(c + 1) * P, 2 * D:3 * D])
            for c in range(DC):
                nc.sync.dma_start(wout_sb[:, c, :], woutT.ap()[c * P:(c + 1) * P, :])
            nc.sync.dma_start(bout_sb[:, :], bout.ap()[:, :])

            ones128 = cp.tile([1, P], BF16)
            nc.vector.memset(ones128, 1.0)
            ones64 = cp.tile([1, DH], F32)
            nc.vector.memset(ones64, 1.0)

            q_sb = cp.tile([P, ET, NQ], BF16)
            k_sb = cp.tile([P, ET, N], BF16)
            v_sb = cp.tile([P, NT, H, DH + 1], BF16)
            nc.vector.memset(v_sb[:, :, :, DH:DH + 1], 1.0)
            aT_sb = cp.tile([P, DC, NQ], BF16)

            def q_proj(et):
                q_ps = pp.tile([P, NQ], F32, tag="proj", bufs=2, name=f"qps{et}")
                for c in range(DC):
                    nc.tensor.matmul(
                        q_ps[:, :],
                        lhsT=wqkv_sb[:, c, et * P:(et + 1) * P],
                        rhs=xq_sb[:, c, :],
                        start=(c == 0), stop=(c == DC - 1),
                    )
                nc.vector.tensor_copy(q_sb[:, et, :], q_ps[:, :])

            def k_proj(et):
                for j in range(2):
                    k_ps = pp.tile([P, NQ], F32, tag="proj", bufs=2, name=f"kps{et}_{j}")
                    for c in range(DC):
                        nc.tensor.matmul(
                            k_ps[:, :],
                            lhsT=wqkv_sb[:, c, D + et * P:D + (et + 1) * P],
                            rhs=xk_sb[:, c, j * NQ:(j + 1) * NQ],
                            start=(c == 0), stop=(c == DC - 1),
                        )
                    nc.vector.tensor_copy(k_sb[:, et, j * NQ:(j + 1) * NQ], k_ps[:, :])

            def v_proj(nt, j):
                v_ps = pp.tile([P, NQ], F32, tag="proj", bufs=2, name=f"vps{nt}_{j}")
                for c in range(DC):
                    nc.tensor.matmul(
                        v_ps[:, :],
                        lhsT=xk_sb[:, c, nt * P:(nt + 1) * P],
                        rhs=wqkv_sb[:, c, 2 * D + j * NQ:2 * D + (j + 1) * NQ],
                        start=(c == 0), stop=(c == DC - 1),
                    )
                nc.vector.tensor_copy(
                    v_sb[:, nt, j * 8:(j + 1) * 8, 0:DH],
                    v_ps[:, :].rearrange("p (h d) -> p h d", h=8),
                )

            # Filler queue: individual projection matmuls threaded between
            # attention matmuls so the in-order PE queue never stalls on the
            # ACT exp pipeline (an idle PE window re-throttles the clock).
            # Each filler step emits one matmul; the 8th closes the group
            # with its PSUM->SBUF eviction.
            def kproj_steps(et, j):
                state = {}
                def step(c):
                    if c == 0:
                        state["ps"] = pp.tile([P, NQ], F32, tag="proj", bufs=2,
                                              name=f"kps{et}_{j}")
                    nc.tensor.matmul(
                        state["ps"][:, :],
                        lhsT=wqkv_sb[:, c, D + et * P:D + (et + 1) * P],
                        rhs=xk_sb[:, c, j * NQ:(j + 1) * NQ],
                        start=(c == 0), stop=(c == DC - 1),
                    )
                    if c == DC - 1:
                        nc.vector.tensor_copy(k_sb[:, et, j * NQ:(j + 1) * NQ],
                                              state["ps"][:, :])
                return [lambda c=c: step(c) for c in range(DC)]

            def vproj_steps(nt, j):
                state = {}
                def step(c):
                    if c == 0:
                        state["ps"] = pp.tile([P, NQ], F32, tag="proj", bufs=2,
                                              name=f"vps{nt}_{j}")
                    nc.tensor.matmul(
                        state["ps"][:, :],
                        lhsT=xk_sb[:, c, nt * P:(nt + 1) * P],
                        rhs=wqkv_sb[:, c, 2 * D + j * NQ:2 * D + (j + 1) * NQ],
                        start=(c == 0), stop=(c == DC - 1),
                    )
                    if c == DC - 1:
                        nc.vector.tensor_copy(
                            v_sb[:, nt, j * 8:(j + 1) * 8, 0:DH],
                            state["ps"][:, :].rearrange("p (h d) -> p h d", h=8),
                        )
                return [lambda c=c: step(c) for c in range(DC)]

            def qproj_steps(et):
                state = {}
                def step(c):
                    if c == 0:
                        state["ps"] = pp.tile([P, NQ], F32, tag="proj", bufs=2,
                                              name=f"qps{et}")
                    nc.tensor.matmul(
                        state["ps"][:, :],
                        lhsT=wqkv_sb[:, c, et * P:(et + 1) * P],
                        rhs=xq_sb[:, c, :],
                        start=(c == 0), stop=(c == DC - 1),
                    )
                    if c == DC - 1:
                        nc.vector.tensor_copy(q_sb[:, et, :], state["ps"][:, :])
                return [lambda c=c: step(c) for c in range(DC)]

            # (deadline, steps): all steps with deadline <= h flush before
            # head h emits its first scores matmul. q-projections for late
            # e-tiles ride in the filler stream too - their deadline is the
            # first head that reads them.
            filler_units = []
            for et in range(1, 4):
                for j in range(2):
                    filler_units.append((2 * et, kproj_steps(et, j)))
            filler_units.append((8, kproj_steps(4, 0)))
            filler_units.append((8, kproj_steps(4, 1)))
            for nt in range(NT):
                filler_units.append((8, vproj_steps(nt, 1)))
            for et in range(5, 8):
                filler_units.append((2 * et, kproj_steps(et, 0)))
                filler_units.append((2 * et, kproj_steps(et, 1)))
            filler_steps = [(dl, s) for dl, steps in filler_units for s in steps]
            fill_pos = 0

            def flush_fillers(h):
                nonlocal fill_pos
                while fill_pos < len(filler_steps) and filler_steps[fill_pos][0] <= h:
                    filler_steps[fill_pos][1]()
                    fill_pos += 1

            def pop_filler(h, n):
                nonlocal fill_pos
                k = 0
                while k < n and fill_pos < len(filler_steps):
                    # never emit a unit earlier than needed relative to others
                    filler_steps[fill_pos][1]()
                    fill_pos += 1
                    k += 1

            # Deferred per-head normalization: the bc outer-product depends on
            # the DVE reciprocal (~3.4us). Emitting it right after PV would
            # stall the in-order PE queue, so it is pushed into the middle of
            # the NEXT head's scores stretch, by which time srec is ready.
            pending_norm = []

            def finish_head(h, t, r, pv_ps, srec):
                bc_ps = pp.tile([P, NQ], F32, tag="bcy", bufs=2, name=f"bc{h}")
                nc.tensor.matmul(bc_ps[0:DH, :], lhsT=ones64[:, :], rhs=srec[:, :],
                                 start=True, stop=True)
                bc_sb = wp.tile([DH, NQ], F32, tag="bc_sb", bufs=2, name=f"bcs{h}")
                nc.vector.tensor_copy(bc_sb[:, :], bc_ps[0:DH, :])
                nc.vector.tensor_mul(aT_sb[r:r + DH, t, :], pv_ps[0:DH, :], bc_sb[:, :])

            def head(h, fill_per_chunk):
                t, r = h // 2, (h % 2) * DH
                flush_fillers(h)
                pT = wp.tile([P, NT, NQ], BF16, tag="pT", bufs=2, name=f"pT{h}")
                pv_ps = pp.tile([DH + 1, NQ], F32, tag="pv", bufs=2, name=f"pv{h}")
                for c in range(NT):
                    s_ps = pp.tile([P, NQ], F32, tag="s", bufs=2, name=f"s{h}_{c}")
                    nc.tensor.matmul(
                        s_ps[:, :],
                        lhsT=k_sb[r:r + DH, t, c * P:(c + 1) * P],
                        rhs=q_sb[r:r + DH, t, :],
                        start=True, stop=True,
                    )
                    nc.scalar.activation(pT[:, c, :], s_ps[:, :], AF.Exp, scale=SCALE)
                    pop_filler(h, fill_per_chunk[c])
                flush_fillers(h + 0.5)
                for c in range(NT):
                    nc.tensor.matmul(
                        pv_ps[:, :],
                        lhsT=v_sb[:, c, h, :],
                        rhs=pT[:, c, :],
                        start=(c == 0), stop=(c == NT - 1),
                    )
                    # previous head's normalization lands here: ~6us after its
                    # PV finished, so its reciprocal is long done
                    if c == 3 and pending_norm:
                        pending_norm.pop(0)()
                srec = wp.tile([1, NQ], F32, tag="srec", bufs=2, name=f"sr{h}")
                nc.vector.reciprocal(srec[:, :], pv_ps[DH:DH + 1, :])
                pending_norm.append(
                    lambda h=h, t=t, r=r, pv_ps=pv_ps, srec=srec:
                        finish_head(h, t, r, pv_ps, srec))

            for et in range(ET):
                q_proj(et)
            k_proj(0)
            for nt in range(NT):
                v_proj(nt, 0)
            pacing = {c: 2 for c in range(NT)}
            pacing_late = {c: (1 if c < 6 else 0) for c in range(NT)}
            for h in range(H):
                head(h, pacing if h < 8 else pacing_late)

            # The last head's normalization threads into the first output
            # group: chunks 0-5 don't touch heads 14/15, giving the final
            # reciprocal time to drain before the c=6/7 matmuls need aT.
            for t4 in range(NQ // P):
                y_sb = wp.tile([P, D], F32, tag="y_sb", bufs=2, name=f"ysb{t4}")
                for j in range(2):
                    y_ps = pp.tile([P, NQ], F32, tag="bcy", bufs=2, name=f"yps{t4}_{j}")
                    nc.tensor.matmul(y_ps[:, :], lhsT=ones128[:, :],
                                     rhs=bout_sb[:, j * NQ:(j + 1) * NQ],
                                     start=True, stop=False)
                    for c in range(DC):
                        if c == 5 and pending_norm:
                            pending_norm.pop(0)()
                        nc.tensor.matmul(
                            y_ps[:, :],
                            lhsT=aT_sb[:, c, t4 * P:(t4 + 1) * P],
                            rhs=wout_sb[:, c, j * NQ:(j + 1) * NQ],
                            start=False, stop=(c == DC - 1),
                        )
                    nc.vector.tensor_copy(y_sb[:, j * NQ:(j + 1) * NQ], y_ps[:, :])
                nc.sync.dma_start(y.ap()[t4 * P:(t4 + 1) * P, :], y_sb[:, :])
    nc.compile()
    return nc


def _make_in_maps(x, w_qkv, w_out, b_out):
    import ml_dtypes
    bf = ml_dtypes.bfloat16
    wqkvT = np.ascontiguousarray(w_qkv.astype(bf).T)
    woutT = np.ascontiguousarray(w_out.astype(bf).T)
    boutr = b_out.astype(bf).reshape(1, D)
    in_maps = []
    for core in range(8):
        b, half = core // 2, core % 2
        xT = np.ascontiguousarray(x[b].astype(bf).T)
        in_maps.append({
            "xkT": xT,
            "xqT": np.ascontiguousarray(xT[:, half * NQ:(half + 1) * NQ]),
            "wqkvT": wqkvT,
            "woutT": woutT,
            "bout": boutr,
        })
    return in_maps


def _assemble(results):
    y = np.empty((B, N, D), dtype=np.float32)
    for core in range(8):
        b, half = core // 2, core % 2
        y[b, half * NQ:(half + 1) * NQ, :] = results[core]["y"]
    return y


_NC_CACHE = {}


def kernel(x, w_qkv, w_out, b_out):
    import numpy as _np
    from concourse.bass_utils import run_bass_kernel_spmd
    if "nc" not in _NC_CACHE:
        _NC_CACHE["nc"] = _build_nc()
    nc = _NC_CACHE["nc"]
    in_maps = _make_in_maps(_np.asarray(x), _np.asarray(w_qkv),
                            _np.asarray(w_out), _np.asarray(b_out))
    res = run_bass_kernel_spmd(nc, in_maps, list(range(8)))
    return _assemble(res.results)



# revision 15
# speedup vs baseline: 1.4387x; 1.4387x over previous
"""Trainium2 Bass kernel: multi-head attention (B=4, N=1024, D=1024, H=16)
distributed over 8 NeuronCores.

Sharding: core = (batch b, head-group hg), hg selecting 8 of the 16 heads.
Each core projects Q/K/V for its 8 heads only (column-parallel w_qkv), runs
attention for those heads over all 1024 queries, and applies the
row-parallel slice of w_out, producing a partial y[1024, 1024] (fp32).
The host sums the two partials per batch and adds the bias.  This removes
the duplicated K/V projection work of a batch/query-half sharding (~20% of
PE columns).

Per-core schedule: after a short preamble (Q/K for head pair 0), head
iterations are software-pipelined: iteration h emits scores(h+1)
interleaved with PV(h) (lagging 3 score-tiles so the deferred
normalization of head h-1 can free the PV psum banks first) while the
Scalar engine runs the exp stream.  Softmax denominators come free from a
ones-column appended to V; their reciprocal runs on the Scalar engine
([1,512] reciprocal costs ~0.7us there vs 3.4us on DVE).
"""

import numpy as np
import concourse.bacc as bacc
import concourse.mybir as mybir
import concourse.tile as tile

dt = mybir.dt
F32, BF16 = dt.float32, dt.bfloat16

B, N, D = 4, 1024, 1024
H, DH = 16, 64
HG = 8              # heads per core
DG = HG * DH        # 512 head dims per core
P = 128
DC = D // P         # 8 contraction chunks over D
NT = N // P         # 8 key-token tiles
ET = DG // P        # 4 feature tiles (head pairs)
SCALE = DH ** -0.5
AF = mybir.ActivationFunctionType


def _build_nc():
    nc = bacc.Bacc("TRN2", target_bir_lowering=False, debug=False)
    xT = nc.dram_tensor("xT", [D, N], BF16, kind="ExternalInput")
    wqT = nc.dram_tensor("wqT", [D, DG], BF16, kind="ExternalInput")
    wkT = nc.dram_tensor("wkT", [D, DG], BF16, kind="ExternalInput")
    wvT = nc.dram_tensor("wvT", [D, DG], BF16, kind="ExternalInput")
    wo = nc.dram_tensor("wo", [DG, D], BF16, kind="ExternalInput")
    y = nc.dram_tensor("y", [N, D], F32, kind="ExternalOutput")

    with tile.TileContext(nc) as tc:
        with (
            tc.tile_pool(name="const", bufs=1) as cp,
            tc.tile_pool(name="work", bufs=2) as wp,
            tc.tile_pool(name="ps", bufs=1, space="PSUM") as pp,
        ):
            xT_sb = cp.tile([P, DC, N], BF16)
            wq_sb = cp.tile([P, DC, DG], BF16)
            wk_sb = cp.tile([P, DC, DG], BF16)
            wv_sb = cp.tile([P, DC, DG], BF16)
            wo_sb = cp.tile([P, ET, D], BF16)

            # DMA order = consumption order: Q/K proj for head pair 0 first.
            nc.sync.dma_start(wq_sb[:, :, 0:P],
                              wqT.ap()[:, 0:P].rearrange("(c p) e -> p c e", p=P))
            nc.sync.dma_start(xT_sb[:, :, 0:512],
                              xT.ap()[:, 0:512].rearrange("(c p) n -> p c n", p=P))
            nc.sync.dma_start(wk_sb[:, :, 0:P],
                              wkT.ap()[:, 0:P].rearrange("(c p) e -> p c e", p=P))
            nc.sync.dma_start(xT_sb[:, :, 512:N],
                              xT.ap()[:, 512:N].rearrange("(c p) n -> p c n", p=P))
            nc.sync.dma_start(wv_sb[:, :, :],
                              wvT.ap().rearrange("(c p) e -> p c e", p=P))
            nc.sync.dma_start(wq_sb[:, :, P:DG],
                              wqT.ap()[:, P:DG].rearrange("(c p) e -> p c e", p=P))
            nc.sync.dma_start(wk_sb[:, :, P:DG],
                              wkT.ap()[:, P:DG].rearrange("(c p) e -> p c e", p=P))
            nc.sync.dma_start(wo_sb[:, :, :],
                              wo.ap().rearrange("(c p) e -> p c e", p=P))

            q_sb = cp.tile([P, ET, N], BF16)
            k_sb = cp.tile([P, ET, N], BF16)
            # V stationary tile is 128 wide: a ones column at index 0 puts the
            # softmax denominator in psum row 0 (reciprocal_approx_fast, a
            # custom DVE op, drops input partition offsets, and psum reads
            # must start at a multiple of 32 anyway); the V dims sit at
            # columns 64..127 so the normalize multiply reads psum rows
            # 64..127 (offset 64 is legal).  Columns 1..63 are zeroed.
            v_sb = cp.tile([P, NT, HG, P], BF16)
            nc.vector.memset(v_sb[:, :, :, 0:1], 1.0)
            nc.vector.memset(v_sb[:, :, :, 1:DH], 0.0)
            aT_sb = cp.tile([P, ET, N], BF16)

            # ---- projection chains, exposed as single-matmul filler steps ----
            def q_steps(et, j):
                st = {}
                def step(c):
                    if c == 0:
                        st["ps"] = pp.tile([P, 512], F32, tag="proj", bufs=2,
                                           name=f"qps{et}_{j}")
                    nc.tensor.matmul(
                        st["ps"][:, :],
                        lhsT=wq_sb[:, c, et * P:(et + 1) * P],
                        rhs=xT_sb[:, c, j * 512:(j + 1) * 512],
                        start=(c == 0), stop=(c == DC - 1),
                    )
                    if c == DC - 1:
                        nc.vector.tensor_copy(q_sb[:, et, j * 512:(j + 1) * 512],
                                              st["ps"][:, :])
                return [lambda c=c: step(c) for c in range(DC)]

            def k_steps(et, j):
                st = {}
                def step(c):
                    if c == 0:
                        st["ps"] = pp.tile([P, 512], F32, tag="proj", bufs=2,
                                           name=f"kps{et}_{j}")
                    nc.tensor.matmul(
                        st["ps"][:, :],
                        lhsT=wk_sb[:, c, et * P:(et + 1) * P],
                        rhs=xT_sb[:, c, j * 512:(j + 1) * 512],
                        start=(c == 0), stop=(c == DC - 1),
                    )
                    if c == DC - 1:
                        nc.vector.tensor_copy(k_sb[:, et, j * 512:(j + 1) * 512],
                                              st["ps"][:, :])
                return [lambda c=c: step(c) for c in range(DC)]

            def v_steps(nt):
                st = {}
                def step(c):
                    if c == 0:
                        st["ps"] = pp.tile([P, DG], F32, tag="proj", bufs=2,
                                           name=f"vps{nt}")
                    nc.tensor.matmul(
                        st["ps"][:, :],
                        lhsT=xT_sb[:, c, nt * P:(nt + 1) * P],
                        rhs=wv_sb[:, c, :],
                        start=(c == 0), stop=(c == DC - 1),
                    )
                    if c == DC - 1:
                        nc.vector.tensor_copy(
                            v_sb[:, nt, :, DH:P],
                            st["ps"][:, :].rearrange("p (h d) -> p h d", h=HG),
                        )
                return [lambda c=c: step(c) for c in range(DC)]

            # Fillers threaded between attention matmuls so the in-order PE
            # queue stays busy while the Scalar exp stream catches up.
            # late list: Q/K for head pair 3 (not needed until iteration 5's
            # scores(6)) rides inside iterations 0-4.
            fillers = []
            for nt in range(NT):
                fillers += v_steps(nt)
            for et in (1, 2):
                for j in range(2):
                    fillers += q_steps(et, j)
                for j in range(2):
                    fillers += k_steps(et, j)
            late = []
            for j in range(2):
                late += k_steps(3, j)
            for j in range(2):
                late += q_steps(3, j)
            fill_pos = [0]
            late_pos = [0]

            def pop_filler(n):
                k = 0
                while k < n and fill_pos[0] < len(fillers):
                    fillers[fill_pos[0]]()
                    fill_pos[0] += 1
                    k += 1

            def pop_late(n):
                k = 0
                while k < n and late_pos[0] < len(late):
                    late[late_pos[0]]()
                    late_pos[0] += 1
                    k += 1

            state = {}

            def new_head(h):
                state[h] = {"pT": wp.tile([P, NT, 2, 512], BF16, tag="pT",
                                          bufs=2, name=f"pT{h}"),
                            "pv": {}}

            def s_tile(h, j, c):
                t, r = h // 2, (h % 2) * DH
                s_ps = pp.tile([P, 512], F32, tag="s", bufs=2, name=f"s{h}_{j}_{c}")
                nc.tensor.matmul(
                    s_ps[:, :],
                    lhsT=k_sb[r:r + DH, t, c * P:(c + 1) * P],
                    rhs=q_sb[r:r + DH, t, j * 512:(j + 1) * 512],
                    start=True, stop=True,
                )
                nc.scalar.activation(state[h]["pT"][:, c, j, :], s_ps[:, :],
                                     AF.Exp, scale=SCALE)

            def pv_link(h, j, c):
                st = state[h]
                if c == 0:
                    st["pv"][j] = pp.tile([P, 512], F32, tag="pv", bufs=2,
                                          name=f"pv{h}_{j}")
                nc.tensor.matmul(
                    st["pv"][j][:, :],
                    lhsT=v_sb[:, c, h, :],
                    rhs=st["pT"][:, c, j, :],
                    start=(c == 0), stop=(c == NT - 1),
                )
                if c == NT - 1:
                    # Normalization, inline as the chain closes:
                    # 1/s on DVE (fast-approx, 18 bits), broadcast over the 64
                    # head dims on the idle GpSimd engine, multiply on DVE.
                    # No PE or Scalar involvement; frees the pv bank promptly.
                    t, r = h // 2, (h % 2) * DH
                    srec = st.setdefault(
                        "srec", wp.tile([1, 2, 512], F32, tag="srec", bufs=2,
                                        name=f"sr{h}"))
                    nc.vector.reciprocal_approx_fast(srec[:, j, :],
                                                     st["pv"][j][0:1, :])
                    bc_sb = wp.tile([DH, 512], F32, tag="bc_sb", bufs=4,
                                    name=f"bcs{h}_{j}")
                    nc.gpsimd.partition_broadcast(bc_sb[:, :], srec[:, j, :])
                    nc.vector.tensor_mul(aT_sb[r:r + DH, t, j * 512:(j + 1) * 512],
                                         st["pv"][j][DH:P, :], bc_sb[:, :])

            # ---- emission ----
            # Preamble: Q/K for head pair 0, then scores(0) with fillers
            # threaded (2 per tile), then the bulk of the remaining
            # projections.
            for s in q_steps(0, 0) + q_steps(0, 1) + k_steps(0, 0) + k_steps(0, 1):
                s()
            new_head(0)
            for j in range(2):
                for c in range(NT):
                    s_tile(0, j, c)
                    pop_filler(2)
            pop_filler(len(fillers))

            # Head iterations.  Iteration h: scores(h+1) tiles drive; pv(h)
            # links lag LAG tiles behind (head h's pv banks are freed by the
            # inline normalization shortly after each chain closes).
            LAG = 2
            for h in range(HG):
                slots = []
                if h + 1 < HG:
                    new_head(h + 1)
                    slots += [("s", h + 1, j, c) for j in range(2) for c in range(NT)]
                pv_slots = [("pv", h, j, c) for j in range(2) for c in range(NT)]
                out = []
                for i, sl in enumerate(slots):
                    out.append(sl)
                    if i >= LAG - 1 and i - (LAG - 1) < len(pv_slots):
                        out.append(pv_slots[i - (LAG - 1)])
                n_done = max(len(slots) - (LAG - 1), 0)
                out += pv_slots[n_done:]
                for sl in out:
                    if sl[0] == "s":
                        s_tile(*sl[1:])
                        if h < 5:
                            pop_late(1)
                    else:
                        pv_link(*sl[1:])

            # Output projection: y partial [1024, 1024] fp32, contracting the
            # 4 feature tiles (all 8 heads of this group).
            for qt in range(N // P):
                y_sb = wp.tile([P, D], F32, tag="y_sb", bufs=2, name=f"ysb{qt}")
                for jE in range(2):
                    y_ps = pp.tile([P, 512], F32, tag="bcy", bufs=2,
                                   name=f"yps{qt}_{jE}")
                    for et in range(ET):
                        nc.tensor.matmul(
                            y_ps[:, :],
                            lhsT=aT_sb[:, et, qt * P:(qt + 1) * P],
                            rhs=wo_sb[:, et, jE * 512:(jE + 1) * 512],
                            start=(et == 0), stop=(et == ET - 1),
                        )
                    nc.vector.tensor_copy(y_sb[:, jE * 512:(jE + 1) * 512], y_ps[:, :])
                nc.sync.dma_start(y.ap()[qt * P:(qt + 1) * P, :], y_sb[:, :])
    nc.compile()
    return nc


def _make_in_maps(x, w_qkv, w_out, b_out):
    import ml_dtypes
    bf = ml_dtypes.bfloat16
    wq, wk, wv = w_qkv[0:D], w_qkv[D:2 * D], w_qkv[2 * D:3 * D]
    in_maps = []
    for core in range(8):
        b, hg = core // 2, core % 2
        s = slice(hg * DG, (hg + 1) * DG)
        in_maps.append({
            "xT": np.ascontiguousarray(x[b].astype(bf).T),
            "wqT": np.ascontiguousarray(wq[s].astype(bf).T),
            "wkT": np.ascontiguousarray(wk[s].astype(bf).T),
            "wvT": np.ascontiguousarray(wv[s].astype(bf).T),
            # reference einsum is 'bnd,ed->bne': w_out columns are the
            # attention-dim (contraction) axis, so the row-parallel slice is
            # columns hg*DG:(hg+1)*DG of w_out, transposed to [DG, D].
            "wo": np.ascontiguousarray(w_out[:, s].T.astype(bf)),
        })
    return in_maps


def _assemble(results, b_out):
    y = np.empty((B, N, D), dtype=np.float32)
    for b in range(B):
        y[b] = results[2 * b]["y"] + results[2 * b + 1]["y"]
    y += b_out.astype(np.float32)
    return y


_NC_CACHE = {}


def kernel(x, w_qkv, w_out, b_out):
    import numpy as _np
    from concourse.bass_utils import run_bass_kernel_spmd
    if "nc" not in _NC_CACHE:
        _NC_CACHE["nc"] = _build_nc()
    nc = _NC_CACHE["nc"]
    in_maps = _make_in_maps(_np.asarray(x), _np.asarray(w_qkv),
                            _np.asarray(w_out), _np.asarray(b_out))
    res = run_bass_kernel_spmd(nc, in_maps, list(range(8)))
    return _assemble(res.results, _np.asarray(b_out))
